# revision 16
# baseline (speedup 1.0000x reference)
"""BitLinear forward kernel for Trainium2 (8 NeuronCores, data-parallel).

y = sign(x) @ (alpha * code)^T + b, with code/alpha the per-row
ternarization of W (BitNet, delta_w = 0.05, delta_a = 0).

Strategy (vs the 474 us DMA-transpose baseline):
  * x is staged to DRAM pre-transposed in bf16 (layout chosen while
    sharding on the host), so the matmul rhs needs no on-device
    transpose and x HBM read traffic is halved.  sign() is one ACT pass
    straight into fp8.
  * The output is computed transposed (yT [o, t], bf16) so the per-row
    alpha is a per-partition scale applied during PSUM eviction; the
    host transposes back.
  * code is computed as Sign(Wc - thr) + Sign(Wc + thr) in {-2, 0, 2}
    (two ACT passes whose accum_out gives den for free); the factor 2
    is folded into the eviction scale.  alpha comes from the Relu
    identity sum(aWc | aWc>=thr) = sum(relu(aWc-thr)) + thr*den.
  * code blocks are transposed on the PE (128x128 identity matmuls) and
    evicted psum->fp8 in one strided DVE copy per W tile - no DRAM
    bounce, no xbar-transpose DMA.
  * Matmul: fp8 DoubleRow (K=256/pass), N=512, back-to-back per PSUM
    bank - measured ~237 ns/matmul (~142 TF/s) on this hardware.
  * Elementwise quantization work is spread over ACT/DVE/Pool so the
    per-W-tile cadence stays ahead of the PE's per-o-tile GEMM cadence.
"""

import sys

for _p in ("/opt/trn_rl_repo", "/opt/trn_rl_repo/concourse"):
    if _p not in sys.path:
        sys.path.insert(0, _p)

import numpy as np

import concourse.bass as bass
import concourse.tile as tile
import concourse.mybir as mybir
from concourse import bacc
from concourse.bass_utils import run_bass_kernel_spmd

B, S, D, O = 4, 4096, 2048, 2048
N_CORES = 8
T = (B * S) // N_CORES      # 2048 token rows per core
DELTA_W = 0.05
P = 128
DP = D // 256               # 8 paired-d slabs (DoubleRow)
OT = O // P                 # 16 output row tiles == W row tiles

F32 = mybir.dt.float32
BF16 = mybir.dt.bfloat16
FP8 = mybir.dt.float8e4

NP_BF16 = mybir.dt.np(BF16)

Alu = mybir.AluOpType
Act = mybir.ActivationFunctionType

_CACHE = {}


def _build(with_bias: bool):
    nc = bacc.Bacc("TRN2", target_bir_lowering=False, debug=False,
                   num_devices=N_CORES)
    xT_d = nc.dram_tensor("xT", [D, T], BF16, kind="ExternalInput").ap()
    w_d = nc.dram_tensor("W", [O, D], F32, kind="ExternalInput").ap()
    eye_d = nc.dram_tensor("eye", [P, P], F32, kind="ExternalInput").ap()
    yT_d = nc.dram_tensor("yT", [O, T], BF16, kind="ExternalOutput").ap()
    if with_bias:
        b_d = nc.dram_tensor("b", [O], F32, kind="ExternalInput").ap()

    with tile.TileContext(nc) as tc:
        with (
            tc.tile_pool(name="wload", bufs=3) as wload,
            tc.tile_pool(name="junk", bufs=1) as junk_pool,
            tc.tile_pool(name="gp", bufs=2) as gpool,
            tc.tile_pool(name="code2", bufs=2) as code2_pool,
            tc.tile_pool(name="stats", bufs=1) as stats,
            tc.tile_pool(name="xstage", bufs=3) as xstage_pool,
            tc.tile_pool(name="xqT", bufs=DP) as xqT_pool,
            tc.tile_pool(name="codeT", bufs=1) as codeT_pool,
            tc.tile_pool(name="small", bufs=1) as small,
            tc.tile_pool(name="ysb", bufs=4) as ysb_pool,
            tc.tile_pool(name="tp_ps", bufs=1, space="PSUM") as tp_ps_pool,
            tc.tile_pool(name="mm_ps", bufs=3 - int(with_bias),
                         space="PSUM") as mm_ps_pool,
        ):
            # identity for PE transposes
            eye_bf = small.tile([P, P], BF16, tag="eyebf")
            nc.gpsimd.dma_start(eye_bf[:], eye_d[:, :])
            eye_f8 = small.tile([P, P], FP8, tag="eyef8")
            nc.vector.tensor_copy(out=eye_f8[:], in_=eye_bf[:])

            # per-row stats, one column per W tile / o-tile
            S_t = stats.tile([P, OT], F32, tag="S")
            negmean = stats.tile([P, OT], F32, tag="negmean")
            T_t = stats.tile([P, OT], F32, tag="T")
            thr = stats.tile([P, OT], F32, tag="thr")
            negthr = stats.tile([P, OT], F32, tag="negthr")
            bp = stats.tile([P, OT], F32, tag="bp")
            bm = stats.tile([P, OT], F32, tag="bm")
            Sg1 = stats.tile([P, OT], F32, tag="Sg1")
            Sg2 = stats.tile([P, OT], F32, tag="Sg2")
            R_t = stats.tile([P, OT], F32, tag="R")
            den = stats.tile([P, OT], F32, tag="den")
            t1 = stats.tile([P, OT], F32, tag="t1")
            num = stats.tile([P, OT], F32, tag="num")
            rden = stats.tile([P, OT], F32, tag="rden")
            scale_a = stats.tile([P, OT], F32, tag="scalea")

            junk_f = junk_pool.tile([P, D], F32, tag="junkf")
            zeros_b = junk_pool.tile([P, D], BF16, tag="zerosb")
            nc.vector.memset(zeros_b[:], 0.0)

            # full transposed ternary code, fp8: free = dt*2048 + o
            codeT = codeT_pool.tile([P, OT * O // P * P], FP8, tag="codeT")
            codeT_v = codeT[:].rearrange("p (dt o) -> p dt o", dt=16)

            # ---- x: load transposed slabs, sign into fp8 (ACT first) ----
            xqT = []
            for dp in range(DP):
                xs = xstage_pool.tile([P, 2 * T], BF16, tag="xs",
                                      name=f"xs_{dp}")
                dma_eng = nc.sync if dp % 2 == 0 else nc.gpsimd
                dma_eng.dma_start(
                    xs[:].rearrange("p (k t) -> p k t", k=2),
                    xT_d[dp * 256:(dp + 1) * 256, :].rearrange(
                        "(k p) t -> p k t", p=P))
                xq = xqT_pool.tile([P, 2 * T], FP8, tag="xqT",
                                   name=f"xqT_{dp}")
                nc.scalar.activation(out=xq[:], in_=xs[:], func=Act.Sign)
                xqT.append(xq)

            # ---- W quantization, one 128-row tile at a time --------------
            for wi in range(OT):
                wt = wload.tile([P, D], F32, tag="wt", name=f"wt_{wi}")
                nc.gpsimd.dma_start(wt[:], w_d[wi * P:(wi + 1) * P, :])
                ws = slice(wi, wi + 1)
                # S = sum(W); mean
                nc.vector.tensor_scalar(
                    out=junk_f[:], in0=wt[:], scalar1=0.0, scalar2=0.0,
                    op0=Alu.add, op1=Alu.add, accum_out=S_t[:, ws])
                nc.vector.tensor_scalar_mul(negmean[:, ws], S_t[:, ws],
                                            -1.0 / D)
                # aWc (bf16) + T = sum |W - mean| on ACT
                aWc = gpool.tile([P, D], BF16, tag="aWc", name=f"aWc_{wi}")
                nc.scalar.activation(
                    out=aWc[:], in_=wt[:], func=Act.Abs,
                    bias=negmean[:, ws], accum_out=T_t[:, ws])
                nc.vector.tensor_scalar_mul(thr[:, ws], T_t[:, ws],
                                            DELTA_W / D)
                nc.vector.tensor_scalar_mul(negthr[:, ws], T_t[:, ws],
                                            -DELTA_W / D)
                # bp = -(mean + thr), bm = -(mean - thr)
                nc.vector.tensor_sub(bp[:, ws], negmean[:, ws], thr[:, ws])
                nc.vector.tensor_add(bm[:, ws], negmean[:, ws], thr[:, ws])
                # g1 = Sign(W - mean - thr), g2 = Sign(W - mean + thr)
                g1 = gpool.tile([P, D], BF16, tag="g1", name=f"g1_{wi}")
                g2 = gpool.tile([P, D], BF16, tag="g2", name=f"g2_{wi}")
                nc.scalar.activation(out=g1[:], in_=wt[:], func=Act.Sign,
                                     bias=bp[:, ws], accum_out=Sg1[:, ws])
                nc.scalar.activation(out=g2[:], in_=wt[:], func=Act.Sign,
                                     bias=bm[:, ws], accum_out=Sg2[:, ws])
                # code2 = g1 + g2 in {-2, 0, 2} (Pool engine, fp8 out)
                code2 = code2_pool.tile([P, D], FP8, tag="code2",
                                        name=f"code2_{wi}")
                nc.gpsimd.tensor_add(code2[:], g1[:], g2[:])
                # R = sum relu(aWc - thr)
                nc.vector.scalar_tensor_tensor(
                    out=g1[:], in0=aWc[:], scalar=negthr[:, ws],
                    in1=zeros_b[:], op0=Alu.add, op1=Alu.max,
                    accum_out=R_t[:, ws])
                # transpose code2 -> psum (bf16), evict fp8 into codeT
                tp = tp_ps_pool.tile([P, 2 * D], FP8, tag="tp",
                                     name=f"tp_{wi}")
                tp_v = tp[:].rearrange("p (n two) -> p n two", two=2)[:, :, 0]
                for dt in range(16):
                    nc.tensor.transpose(
                        tp_v[:, dt * P:(dt + 1) * P],
                        code2[:, dt * P:(dt + 1) * P], eye_f8[:])
                nc.vector.tensor_copy(
                    out=codeT_v[:, :, wi * P:(wi + 1) * P],
                    in_=tp_v.rearrange("p (dt o) -> p dt o", dt=16))

            # den = D + (Sg1 - Sg2)/2 clipped at 1
            nc.vector.tensor_sub(t1[:], Sg1[:], Sg2[:])
            nc.vector.tensor_scalar(
                out=den[:], in0=t1[:], scalar1=0.5, scalar2=float(D),
                op0=Alu.mult, op1=Alu.add)
            nc.vector.tensor_scalar_max(den[:], den[:], 1.0)
            # eviction scale = alpha/2 = (R + thr*den) / (2*den)
            nc.vector.tensor_mul(num[:], thr[:], den[:])
            nc.vector.tensor_add(num[:], num[:], R_t[:])
            nc.vector.reciprocal(rden[:], den[:])
            nc.vector.tensor_mul(scale_a[:], num[:], rden[:])
            nc.vector.tensor_scalar_mul(scale_a[:], scale_a[:], 0.5)

            if with_bias:
                eye16 = small.tile([16, 16], F32, tag="eye16")
                nc.gpsimd.dma_start(eye16[:], eye_d[0:16, 0:16])
                b16 = small.tile([16, P], F32, tag="b16")
                nc.sync.dma_start(b16[:],
                                  b_d[:].rearrange("(j p) -> j p", j=16))
                b_ps = tp_ps_pool.tile([P, 16], F32, tag="bps")
                nc.tensor.transpose(b_ps[:], b16[:], eye16[:])
                bias_sb = small.tile([P, 16], F32, tag="biassb")
                nc.vector.tensor_copy(out=bias_sb[:], in_=b_ps[:])

            # ---- main matmul: yT[o, t] = codeT^T @ xqT ------------------
            for j in range(OT):
                for hh in range(2):
                    ps = mm_ps_pool.tile([P, T // 2], F32, tag="ps",
                                         name=f"ps{j}_{hh}")
                    for bk in range(2):
                        t0 = hh * (T // 2) + bk * 512
                        for dp in range(DP):
                            lhsT = codeT_v[:, 2 * dp:2 * dp + 2,
                                           j * P:(j + 1) * P]
                            rhs = xqT[dp][:].rearrange(
                                "p (k t) -> p k t", k=2)[:, :, t0:t0 + 512]
                            nc.tensor.matmul(
                                ps[:, bk * 512:(bk + 1) * 512], lhsT, rhs,
                                start=(dp == 0), stop=(dp == DP - 1),
                                perf_mode=mybir.MatmulPerfMode.DoubleRow)
                    ysb = ysb_pool.tile([P, T // 2], BF16, tag="ysb",
                                        name=f"ysb{j}_{hh}")
                    nc.scalar.activation(
                        out=ysb[:], in_=ps[:], func=Act.Copy,
                        scale=scale_a[:, j:j + 1],
                        bias=(bias_sb[:, j:j + 1] if with_bias else 0.0))
                    nc.scalar.dma_start(
                        yT_d[j * P:(j + 1) * P,
                             hh * (T // 2):(hh + 1) * (T // 2)], ysb[:])

    nc.compile()
    return nc


def _get_nc(with_bias: bool):
    key = with_bias
    if key not in _CACHE:
        _CACHE[key] = _build(with_bias)
    return _CACHE[key]


def _build_in_maps(x: np.ndarray, W: np.ndarray, b: np.ndarray,
                   with_bias: bool):
    xf = x.reshape(B * S, D)
    eye = np.eye(P, dtype=np.float32)
    in_maps = []
    for c in range(N_CORES):
        m = {
            "xT": np.ascontiguousarray(xf[c * T:(c + 1) * T].T).astype(
                NP_BF16),
            "W": W,
            "eye": eye,
        }
        if with_bias:
            m["b"] = b
        in_maps.append(m)
    return in_maps


def kernel(x: np.ndarray, W: np.ndarray, b: np.ndarray) -> np.ndarray:
    x = np.asarray(x, dtype=np.float32)
    W = np.ascontiguousarray(np.asarray(W, dtype=np.float32))
    b = np.asarray(b, dtype=np.float32)
    with_bias = bool(np.any(b))

    nc = _get_nc(with_bias)
    in_maps = _build_in_maps(x, W, b, with_bias)
    res = run_bass_kernel_spmd(nc, in_maps, core_ids=list(range(N_CORES)))
    y = np.concatenate(
        [res.results[c]["yT"].astype(np.float32).T for c in range(N_CORES)],
        axis=0)
    return np.ascontiguousarray(y.reshape(B, S, O))


if __name__ == "__main__":
    rng = np.random.default_rng(0)
    x = rng.standard_normal((B, S, D), dtype=np.float32)
    W = (rng.standard_normal((O, D)) * 0.03).astype(np.float32)
    b = np.zeros((O,), dtype=np.float32)
    y = kernel(x, W, b)
    print("kernel ran, y shape", y.shape, "mean|y|", np.abs(y).mean())


# revision 18
# speedup vs baseline: 1.2118x; 1.2118x over previous
"""BitLinear forward kernel for Trainium2 (8 NeuronCores, data-parallel).

y = sign(x) @ (alpha * code)^T + b, with code/alpha the per-row
ternarization of W (BitNet, delta_w = 0.05, delta_a = 0).

Strategy (vs the 474 us DMA-transpose baseline):
  * x is staged to DRAM pre-transposed in bf16 (layout chosen while
    sharding on the host), so the matmul rhs needs no on-device
    transpose and x HBM read traffic is halved.  sign() is one ACT pass
    straight into fp8.
  * The output is computed transposed (yT [o, t], bf16) so the per-row
    alpha is a per-partition scale applied during PSUM eviction; the
    host transposes back.
  * code is computed as Sign(Wc - thr) + Sign(Wc + thr) in {-2, 0, 2}
    (two ACT passes whose accum_out gives den for free); the factor 2
    is folded into the eviction scale.  alpha comes from the Relu
    identity sum(aWc | aWc>=thr) = sum(relu(aWc-thr)) + thr*den.
  * code blocks are transposed on the PE (128x128 identity matmuls) and
    evicted psum->fp8 in one strided DVE copy per W tile - no DRAM
    bounce, no xbar-transpose DMA.
  * Matmul: fp8 DoubleRow (K=256/pass), N=512, back-to-back per PSUM
    bank - measured ~237 ns/matmul (~142 TF/s) on this hardware.
  * Elementwise quantization work is spread over ACT/DVE/Pool so the
    per-W-tile cadence stays ahead of the PE's per-o-tile GEMM cadence.
"""

import sys

for _p in ("/opt/trn_rl_repo", "/opt/trn_rl_repo/concourse"):
    if _p not in sys.path:
        sys.path.insert(0, _p)

import numpy as np

import concourse.bass as bass
import concourse.tile as tile
import concourse.mybir as mybir
from concourse import bacc
from concourse.bass_utils import run_bass_kernel_spmd

B, S, D, O = 4, 4096, 2048, 2048
N_CORES = 8
T = (B * S) // N_CORES      # 2048 token rows per core
DELTA_W = 0.05
P = 128
DP = D // 256               # 8 paired-d slabs (DoubleRow)
OT = O // P                 # 16 output row tiles == W row tiles

F32 = mybir.dt.float32
BF16 = mybir.dt.bfloat16
FP8 = mybir.dt.float8e4

NP_BF16 = mybir.dt.np(BF16)

Alu = mybir.AluOpType
Act = mybir.ActivationFunctionType

_CACHE = {}


def _build(with_bias: bool):
    nc = bacc.Bacc("TRN2", target_bir_lowering=False, debug=False,
                   num_devices=N_CORES)
    xT_d = nc.dram_tensor("xT", [D, T], BF16, kind="ExternalInput").ap()
    w_d = nc.dram_tensor("W", [O, D], F32, kind="ExternalInput").ap()
    eye_d = nc.dram_tensor("eye", [P, P], F32, kind="ExternalInput").ap()
    yT_d = nc.dram_tensor("yT", [O, T], BF16, kind="ExternalOutput").ap()
    if with_bias:
        b_d = nc.dram_tensor("b", [O], F32, kind="ExternalInput").ap()

    with tile.TileContext(nc) as tc:
        with (
            tc.tile_pool(name="wload", bufs=3) as wload,
            tc.tile_pool(name="junk", bufs=1) as junk_pool,
            tc.tile_pool(name="gp", bufs=2) as gpool,
            tc.tile_pool(name="code2", bufs=3) as code2_pool,
            tc.tile_pool(name="stats", bufs=1) as stats,
            tc.tile_pool(name="xstage", bufs=DP) as xstage_pool,
            tc.tile_pool(name="xqT", bufs=DP) as xqT_pool,
            tc.tile_pool(name="codeT", bufs=1) as codeT_pool,
            tc.tile_pool(name="small", bufs=1) as small,
            tc.tile_pool(name="ysb", bufs=4) as ysb_pool,
            tc.tile_pool(name="tp_ps", bufs=1, space="PSUM") as tp_ps_pool,
            tc.tile_pool(name="mm_ps", bufs=3 - int(with_bias),
                         space="PSUM") as mm_ps_pool,
        ):
            # identity for PE transposes
            eye_bf = small.tile([P, P], BF16, tag="eyebf")
            nc.gpsimd.dma_start(eye_bf[:], eye_d[:, :])
            eye_f8 = small.tile([P, P], FP8, tag="eyef8")
            nc.vector.tensor_copy(out=eye_f8[:], in_=eye_bf[:])

            # per-row stats, one column per W tile / o-tile
            S_t = stats.tile([P, OT], F32, tag="S")
            negmean = stats.tile([P, OT], F32, tag="negmean")
            T_t = stats.tile([P, OT], F32, tag="T")
            thr = stats.tile([P, OT], F32, tag="thr")
            negthr = stats.tile([P, OT], F32, tag="negthr")
            bp = stats.tile([P, OT], F32, tag="bp")
            bm = stats.tile([P, OT], F32, tag="bm")
            Sg1 = stats.tile([P, OT], F32, tag="Sg1")
            Sg2 = stats.tile([P, OT], F32, tag="Sg2")
            R_t = stats.tile([P, OT], F32, tag="R")
            den = stats.tile([P, OT], F32, tag="den")
            t1 = stats.tile([P, OT], F32, tag="t1")
            num = stats.tile([P, OT], F32, tag="num")
            rden = stats.tile([P, OT], F32, tag="rden")
            scale_a = stats.tile([P, OT], F32, tag="scalea")

            junk_f = junk_pool.tile([P, D], F32, tag="junkf")
            zeros_b = junk_pool.tile([P, D], BF16, tag="zerosb")
            nc.vector.memset(zeros_b[:], 0.0)

            # full transposed ternary code, fp8: free = dt*2048 + o
            codeT = codeT_pool.tile([P, OT * O // P * P], FP8, tag="codeT")
            codeT_v = codeT[:].rearrange("p (dt o) -> p dt o", dt=16)

            # ---- x: start all transposed-slab DMA loads up front ---------
            xstages = []
            xqT = []
            for dp in range(DP):
                xs = xstage_pool.tile([P, 2 * T], BF16, tag="xs",
                                      name=f"xs_{dp}")
                dma_eng = nc.sync if dp % 2 == 0 else nc.scalar
                dma_eng.dma_start(
                    xs[:].rearrange("p (k t) -> p k t", k=2),
                    xT_d[dp * 256:(dp + 1) * 256, :].rearrange(
                        "(k p) t -> p k t", p=P))
                xstages.append(xs)
                xq = xqT_pool.tile([P, 2 * T], FP8, tag="xqT",
                                   name=f"xqT_{dp}")
                xqT.append(xq)

            def emit_xq_sign(dp):
                nc.scalar.activation(out=xqT[dp][:], in_=xstages[dp][:],
                                     func=Act.Sign)

            # two signs early (their DMAs land first), rest interleaved below
            emit_xq_sign(0)
            emit_xq_sign(1)

            # ---- W quantization, one 128-row tile at a time --------------
            for wi in range(OT):
                wt = wload.tile([P, D], F32, tag="wt", name=f"wt_{wi}")
                nc.gpsimd.dma_start(wt[:], w_d[wi * P:(wi + 1) * P, :])
                ws = slice(wi, wi + 1)
                # S = sum(W); mean
                nc.vector.tensor_scalar(
                    out=junk_f[:], in0=wt[:], scalar1=0.0, scalar2=0.0,
                    op0=Alu.add, op1=Alu.add, accum_out=S_t[:, ws])
                nc.vector.tensor_scalar_mul(negmean[:, ws], S_t[:, ws],
                                            -1.0 / D)
                # aWc (bf16) + T = sum |W - mean| on ACT
                aWc = gpool.tile([P, D], BF16, tag="aWc", name=f"aWc_{wi}")
                nc.scalar.activation(
                    out=aWc[:], in_=wt[:], func=Act.Abs,
                    bias=negmean[:, ws], accum_out=T_t[:, ws])
                nc.vector.tensor_scalar_mul(thr[:, ws], T_t[:, ws],
                                            DELTA_W / D)
                nc.vector.tensor_scalar_mul(negthr[:, ws], T_t[:, ws],
                                            -DELTA_W / D)
                # bp = -(mean + thr), bm = -(mean - thr)
                nc.vector.tensor_sub(bp[:, ws], negmean[:, ws], thr[:, ws])
                nc.vector.tensor_add(bm[:, ws], negmean[:, ws], thr[:, ws])
                # g1 = Sign(W - mean - thr), g2 = Sign(W - mean + thr)
                g1 = gpool.tile([P, D], BF16, tag="g1", name=f"g1_{wi}")
                g2 = gpool.tile([P, D], BF16, tag="g2", name=f"g2_{wi}")
                nc.scalar.activation(out=g1[:], in_=wt[:], func=Act.Sign,
                                     bias=bp[:, ws], accum_out=Sg1[:, ws])
                nc.scalar.activation(out=g2[:], in_=wt[:], func=Act.Sign,
                                     bias=bm[:, ws], accum_out=Sg2[:, ws])
                # code2 = g1 + g2 in {-2, 0, 2} (Pool engine, fp8 out)
                code2 = code2_pool.tile([P, D], FP8, tag="code2",
                                        name=f"code2_{wi}")
                nc.gpsimd.tensor_add(code2[:], g1[:], g2[:])
                # R = sum relu(aWc - thr)
                nc.vector.scalar_tensor_tensor(
                    out=g1[:], in0=aWc[:], scalar=negthr[:, ws],
                    in1=zeros_b[:], op0=Alu.add, op1=Alu.max,
                    accum_out=R_t[:, ws])
                if wi < 6:
                    emit_xq_sign(2 + wi)
                # per-tile eviction scale so o-tile wi can evict early:
                # den = max(D + (Sg1-Sg2)/2, 1);  scale = (R+thr*den)/(2*den)
                nc.gpsimd.tensor_sub(t1[:, ws], Sg1[:, ws], Sg2[:, ws])
                nc.vector.tensor_scalar(
                    out=den[:, ws], in0=t1[:, ws], scalar1=0.5,
                    scalar2=float(D), op0=Alu.mult, op1=Alu.add)
                nc.vector.tensor_scalar_max(den[:, ws], den[:, ws], 1.0)
                nc.vector.tensor_mul(num[:, ws], thr[:, ws], den[:, ws])
                nc.vector.tensor_add(num[:, ws], num[:, ws], R_t[:, ws])
                nc.vector.reciprocal(rden[:, ws], den[:, ws])
                nc.vector.tensor_mul(scale_a[:, ws], num[:, ws], rden[:, ws])
                nc.vector.tensor_scalar_mul(scale_a[:, ws], scale_a[:, ws],
                                            0.5)
                # transpose code2 -> psum (bf16), evict fp8 into codeT
                tp = tp_ps_pool.tile([P, 2 * D], FP8, tag="tp",
                                     name=f"tp_{wi}")
                tp_v = tp[:].rearrange("p (n two) -> p n two", two=2)[:, :, 0]
                for dt in range(16):
                    nc.tensor.transpose(
                        tp_v[:, dt * P:(dt + 1) * P],
                        code2[:, dt * P:(dt + 1) * P], eye_f8[:])
                nc.vector.tensor_copy(
                    out=codeT_v[:, :, wi * P:(wi + 1) * P],
                    in_=tp_v.rearrange("p (dt o) -> p dt o", dt=16))

            if with_bias:
                eye16 = small.tile([16, 16], F32, tag="eye16")
                nc.gpsimd.dma_start(eye16[:], eye_d[0:16, 0:16])
                b16 = small.tile([16, P], F32, tag="b16")
                nc.sync.dma_start(b16[:],
                                  b_d[:].rearrange("(j p) -> j p", j=16))
                b_ps = tp_ps_pool.tile([P, 16], F32, tag="bps")
                nc.tensor.transpose(b_ps[:], b16[:], eye16[:])
                bias_sb = small.tile([P, 16], F32, tag="biassb")
                nc.vector.tensor_copy(out=bias_sb[:], in_=b_ps[:])

            # ---- main matmul: yT[o, t] = codeT^T @ xqT ------------------
            for j in range(OT):
                for hh in range(2):
                    ps = mm_ps_pool.tile([P, T // 2], F32, tag="ps",
                                         name=f"ps{j}_{hh}")
                    for bk in range(2):
                        t0 = hh * (T // 2) + bk * 512
                        for dp in range(DP):
                            lhsT = codeT_v[:, 2 * dp:2 * dp + 2,
                                           j * P:(j + 1) * P]
                            rhs = xqT[dp][:].rearrange(
                                "p (k t) -> p k t", k=2)[:, :, t0:t0 + 512]
                            nc.tensor.matmul(
                                ps[:, bk * 512:(bk + 1) * 512], lhsT, rhs,
                                start=(dp == 0), stop=(dp == DP - 1),
                                perf_mode=mybir.MatmulPerfMode.DoubleRow)
                    ysb = ysb_pool.tile([P, T // 2], BF16, tag="ysb",
                                        name=f"ysb{j}_{hh}")
                    nc.scalar.activation(
                        out=ysb[:], in_=ps[:], func=Act.Copy,
                        scale=scale_a[:, j:j + 1],
                        bias=(bias_sb[:, j:j + 1] if with_bias else 0.0))
                    nc.scalar.dma_start(
                        yT_d[j * P:(j + 1) * P,
                             hh * (T // 2):(hh + 1) * (T // 2)], ysb[:])

    nc.compile()
    return nc


def _get_nc(with_bias: bool):
    key = with_bias
    if key not in _CACHE:
        _CACHE[key] = _build(with_bias)
    return _CACHE[key]


def _build_in_maps(x: np.ndarray, W: np.ndarray, b: np.ndarray,
                   with_bias: bool):
    xf = x.reshape(B * S, D)
    eye = np.eye(P, dtype=np.float32)
    in_maps = []
    for c in range(N_CORES):
        m = {
            "xT": np.ascontiguousarray(xf[c * T:(c + 1) * T].T).astype(
                NP_BF16),
            "W": W,
            "eye": eye,
        }
        if with_bias:
            m["b"] = b
        in_maps.append(m)
    return in_maps


def kernel(x: np.ndarray, W: np.ndarray, b: np.ndarray) -> np.ndarray:
    x = np.asarray(x, dtype=np.float32)
    W = np.ascontiguousarray(np.asarray(W, dtype=np.float32))
    b = np.asarray(b, dtype=np.float32)
    with_bias = bool(np.any(b))

    nc = _get_nc(with_bias)
    in_maps = _build_in_maps(x, W, b, with_bias)
    res = run_bass_kernel_spmd(nc, in_maps, core_ids=list(range(N_CORES)))
    y = np.concatenate(
        [res.results[c]["yT"].astype(np.float32).T for c in range(N_CORES)],
        axis=0)
    return np.ascontiguousarray(y.reshape(B, S, O))


if __name__ == "__main__":
    rng = np.random.default_rng(0)
    x = rng.standard_normal((B, S, D), dtype=np.float32)
    W = (rng.standard_normal((O, D)) * 0.03).astype(np.float32)
    b = np.zeros((O,), dtype=np.float32)
    y = kernel(x, W, b)
    print("kernel ran, y shape", y.shape, "mean|y|", np.abs(y).mean())


# revision 21
# speedup vs baseline: 1.2411x; 1.0242x over previous
"""BitLinear forward kernel for Trainium2 (8 NeuronCores, data-parallel).

y = sign(x) @ (alpha * code)^T + b, with code/alpha the per-row
ternarization of W (BitNet, delta_w = 0.05, delta_a = 0).

Strategy (vs the 474 us DMA-transpose baseline):
  * x is staged to DRAM pre-transposed in bf16 (layout chosen while
    sharding on the host): no on-device transpose of x, half the read
    traffic.  sign() is one ACT pass straight into fp8.
  * Output computed transposed (yT [o, t], bf16): per-row alpha becomes
    a per-partition scale applied during PSUM eviction; host transposes
    back.
  * code2 = Sign(Wc - thr) + Sign(Wc + thr) in {-2, 0, 2}; the /2 is
    folded into the eviction scale.  accum_out of the two Sign passes
    gives den; alpha numerator via sum(relu(aWc - thr)) + thr*den.
  * code blocks transposed on the PE (identity matmuls, fp8), evicted
    psum->SBUF in one strided copy per W tile.  No DRAM bounce.
  * Matmul: fp8 DoubleRow, N=512, K=256/pass (~237 ns/matmul measured).
  * Work is emitted in waves of 4 W tiles, phase-by-phase, so each
    engine streams identical ops back-to-back and the cross-engine
    dependency chain pipelines ~4 deep; the GEMM + eviction for those
    4 o-tiles is emitted right after its wave, keeping PE/ACT/DVE all
    running.  Per-tile alpha is finalized inside the wave so evictions
    never wait on later tiles.
"""

import sys

for _p in ("/opt/trn_rl_repo", "/opt/trn_rl_repo/concourse"):
    if _p not in sys.path:
        sys.path.insert(0, _p)

import numpy as np

import concourse.bass as bass
import concourse.tile as tile
import concourse.mybir as mybir
from concourse import bacc
from concourse.bass_utils import run_bass_kernel_spmd

B, S, D, O = 4, 4096, 2048, 2048
N_CORES = 8
T = (B * S) // N_CORES      # 2048 token rows per core
DELTA_W = 0.05
P = 128
DP = D // 256               # 8 paired-d slabs (DoubleRow)
OT = O // P                 # 16 output row tiles == W row tiles
WAVE = 4

F32 = mybir.dt.float32
BF16 = mybir.dt.bfloat16
FP8 = mybir.dt.float8e4

NP_BF16 = mybir.dt.np(BF16)

Alu = mybir.AluOpType
Act = mybir.ActivationFunctionType

_CACHE = {}


def _build(with_bias: bool):
    nc = bacc.Bacc("TRN2", target_bir_lowering=False, debug=False,
                   num_devices=N_CORES)
    xT_d = nc.dram_tensor("xT", [D, T], BF16, kind="ExternalInput").ap()
    w_d = nc.dram_tensor("W", [O, D], F32, kind="ExternalInput").ap()
    eye_d = nc.dram_tensor("eye", [P, P], F32, kind="ExternalInput").ap()
    yT_d = nc.dram_tensor("yT", [O, T], BF16, kind="ExternalOutput").ap()
    if with_bias:
        b_d = nc.dram_tensor("b", [O], F32, kind="ExternalInput").ap()

    with tile.TileContext(nc) as tc:
        with (
            tc.tile_pool(name="wload", bufs=4) as wload,
            tc.tile_pool(name="junk", bufs=1) as junk_pool,
            tc.tile_pool(name="awc", bufs=4) as awc_pool,
            tc.tile_pool(name="gp", bufs=3) as gpool,
            tc.tile_pool(name="code2", bufs=3) as code2_pool,
            tc.tile_pool(name="stats", bufs=1) as stats,
            tc.tile_pool(name="xstage", bufs=10) as xstage_pool,
            tc.tile_pool(name="xqT", bufs=DP) as xqT_pool,
            tc.tile_pool(name="codeT", bufs=1) as codeT_pool,
            tc.tile_pool(name="small", bufs=1) as small,
            tc.tile_pool(name="ysb", bufs=4) as ysb_pool,
            tc.tile_pool(name="tp_ps", bufs=1, space="PSUM") as tp_ps_pool,
            tc.tile_pool(name="mm_ps", bufs=3 - int(with_bias),
                         space="PSUM") as mm_ps_pool,
        ):
            # identity for PE transposes
            eye_bf = small.tile([P, P], BF16, tag="eyebf")
            nc.gpsimd.dma_start(eye_bf[:], eye_d[:, :])
            eye_f8 = small.tile([P, P], FP8, tag="eyef8")
            nc.vector.tensor_copy(out=eye_f8[:], in_=eye_bf[:])

            # per-row stats, one column per W tile / o-tile
            S_t = stats.tile([P, OT], F32, tag="S")
            negmean = stats.tile([P, OT], F32, tag="negmean")
            T_t = stats.tile([P, OT], F32, tag="T")
            thr = stats.tile([P, OT], F32, tag="thr")
            negthr = stats.tile([P, OT], F32, tag="negthr")
            bp = stats.tile([P, OT], F32, tag="bp")
            bm = stats.tile([P, OT], F32, tag="bm")
            Sg1 = stats.tile([P, OT], F32, tag="Sg1")
            Sg2 = stats.tile([P, OT], F32, tag="Sg2")
            R_t = stats.tile([P, OT], F32, tag="R")
            den = stats.tile([P, OT], F32, tag="den")
            t1 = stats.tile([P, OT], F32, tag="t1")
            num = stats.tile([P, OT], F32, tag="num")
            rden = stats.tile([P, OT], F32, tag="rden")
            scale_a = stats.tile([P, OT], F32, tag="scalea")

            junk_b = junk_pool.tile([P, D], BF16, tag="junkb")
            zeros_b = junk_pool.tile([P, D], BF16, tag="zerosb")
            nc.vector.memset(zeros_b[:], 0.0)

            # full transposed ternary code, fp8: free = dt*2048 + o
            codeT = codeT_pool.tile([P, OT * O // P * P], FP8, tag="codeT")
            codeT_v = codeT[:].rearrange("p (dt o) -> p dt o", dt=16)

            # ---- x: all transposed half-slab DMA loads up front ---------
            xstages = {}
            xqT = []
            for dp in range(DP):
                xq = xqT_pool.tile([P, 2 * T], FP8, tag="xqT",
                                   name=f"xqT_{dp}")
                xqT.append(xq)
                for hx in range(2):
                    xs = xstage_pool.tile([P, 2 * (T // 2)], BF16, tag="xs",
                                          name=f"xs_{dp}_{hx}")
                    dma_eng = nc.sync if dp % 2 == 0 else nc.scalar
                    dma_eng.dma_start(
                        xs[:].rearrange("p (k t) -> p k t", k=2),
                        xT_d[dp * 256:(dp + 1) * 256,
                             hx * (T // 2):(hx + 1) * (T // 2)].rearrange(
                            "(k p) t -> p k t", p=P))
                    xstages[(dp, hx)] = xs

            def emit_xq_sign(dp, hx):
                out_v = xqT[dp][:].rearrange("p (k t) -> p k t", k=2)[
                    :, :, hx * (T // 2):(hx + 1) * (T // 2)]
                in_v = xstages[(dp, hx)][:].rearrange("p (k t) -> p k t",
                                                      k=2)
                nc.scalar.activation(out=out_v, in_=in_v, func=Act.Sign)

            # all signs first: every GEMM matmul needs the full xqT set
            for dp in range(DP):
                for hx in range(2):
                    emit_xq_sign(dp, hx)

            if with_bias:
                eye16 = small.tile([16, 16], F32, tag="eye16")
                nc.gpsimd.dma_start(eye16[:], eye_d[0:16, 0:16])
                b16 = small.tile([16, P], F32, tag="b16")
                nc.sync.dma_start(b16[:],
                                  b_d[:].rearrange("(j p) -> j p", j=16))
                b_ps = mm_ps_pool.tile([P, 16], F32, tag="bps")
                nc.tensor.transpose(b_ps[:], b16[:], eye16[:])
                bias_sb = small.tile([P, 16], F32, tag="biassb")
                nc.vector.tensor_copy(out=bias_sb[:], in_=b_ps[:])

            # ---- all W tile loads up front (queue streams them) ---------
            wts = []
            for wi in range(OT):
                wt = wload.tile([P, D], F32, tag="wt", name=f"wt_{wi}")
                nc.gpsimd.dma_start(wt[:], w_d[wi * P:(wi + 1) * P, :])
                wts.append(wt)

            def gemm_otile(j):
                ctv = codeT_v[:, :, j * P:(j + 1) * P]
                for hh in range(2):
                    ps = mm_ps_pool.tile([P, T // 2], F32, tag="ps",
                                         name=f"ps{j}_{hh}")
                    for bk in range(2):
                        t0 = hh * (T // 2) + bk * 512
                        for dp in range(DP):
                            lhsT = ctv[:, 2 * dp:2 * dp + 2, :]
                            rhs = xqT[dp][:].rearrange(
                                "p (k t) -> p k t", k=2)[:, :, t0:t0 + 512]
                            nc.tensor.matmul(
                                ps[:, bk * 512:(bk + 1) * 512], lhsT, rhs,
                                start=(dp == 0), stop=(dp == DP - 1),
                                perf_mode=mybir.MatmulPerfMode.DoubleRow)
                    ysb = ysb_pool.tile([P, T // 2], BF16, tag="ysb",
                                        name=f"ysb{j}_{hh}")
                    if (j + hh) % 2 == 0:
                        nc.scalar.activation(
                            out=ysb[:], in_=ps[:], func=Act.Copy,
                            scale=scale_a[:, j:j + 1],
                            bias=(bias_sb[:, j:j + 1] if with_bias else 0.0))
                    else:
                        if with_bias:
                            nc.vector.tensor_scalar(
                                out=ysb[:], in0=ps[:],
                                scalar1=scale_a[:, j:j + 1],
                                scalar2=bias_sb[:, j:j + 1],
                                op0=Alu.mult, op1=Alu.add)
                        else:
                            nc.vector.tensor_scalar_mul(
                                ysb[:], ps[:], scale_a[:, j:j + 1])
                    nc.sync.dma_start(
                        yT_d[j * P:(j + 1) * P,
                             hh * (T // 2):(hh + 1) * (T // 2)], ysb[:])

            # ---- W quantization in waves, phase-streamed ----------------
            aWcs, g1s, g2s, code2s = {}, {}, {}, {}
            for wave in range(OT // WAVE):
                tiles = range(wave * WAVE, (wave + 1) * WAVE)
                # S + negmean (DVE)
                for wi in tiles:
                    ws = slice(wi, wi + 1)
                    nc.vector.tensor_scalar(
                        out=junk_b[:], in0=wts[wi][:], scalar1=0.0,
                        scalar2=0.0, op0=Alu.add, op1=Alu.add,
                        accum_out=S_t[:, ws])
                    nc.vector.tensor_scalar_mul(negmean[:, ws], S_t[:, ws],
                                                -1.0 / D)
                # aWc + T (ACT)
                for wi in tiles:
                    ws = slice(wi, wi + 1)
                    aWc = awc_pool.tile([P, D], BF16, tag="aWc",
                                        name=f"aWc_{wi}")
                    nc.scalar.activation(
                        out=aWc[:], in_=wts[wi][:], func=Act.Abs,
                        bias=negmean[:, ws], accum_out=T_t[:, ws])
                    aWcs[wi] = aWc
                # thr/negthr (DVE), bp/bm (Pool)
                for wi in tiles:
                    ws = slice(wi, wi + 1)
                    nc.vector.tensor_scalar_mul(thr[:, ws], T_t[:, ws],
                                                DELTA_W / D)
                    nc.vector.tensor_scalar_mul(negthr[:, ws], T_t[:, ws],
                                                -DELTA_W / D)
                    nc.gpsimd.tensor_sub(bp[:, ws], negmean[:, ws],
                                         thr[:, ws])
                    nc.gpsimd.tensor_add(bm[:, ws], negmean[:, ws],
                                         thr[:, ws])
                # g1, g2 (ACT)
                for wi in tiles:
                    ws = slice(wi, wi + 1)
                    g1 = gpool.tile([P, D], BF16, tag="g1", name=f"g1_{wi}")
                    g2 = gpool.tile([P, D], BF16, tag="g2", name=f"g2_{wi}")
                    nc.scalar.activation(out=g1[:], in_=wts[wi][:],
                                         func=Act.Sign, bias=bp[:, ws],
                                         accum_out=Sg1[:, ws])
                    nc.scalar.activation(out=g2[:], in_=wts[wi][:],
                                         func=Act.Sign, bias=bm[:, ws],
                                         accum_out=Sg2[:, ws])
                    g1s[wi], g2s[wi] = g1, g2
                # code2 (Pool, fp8 out)
                for wi in tiles:
                    code2 = code2_pool.tile([P, D], FP8, tag="code2",
                                            name=f"code2_{wi}")
                    nc.gpsimd.tensor_add(code2[:], g1s[wi][:], g2s[wi][:])
                    code2s[wi] = code2
                # R (DVE) + per-tile eviction scale
                for wi in tiles:
                    ws = slice(wi, wi + 1)
                    nc.vector.scalar_tensor_tensor(
                        out=junk_b[:], in0=aWcs[wi][:],
                        scalar=negthr[:, ws], in1=zeros_b[:],
                        op0=Alu.add, op1=Alu.max, accum_out=R_t[:, ws])
                    nc.gpsimd.tensor_sub(t1[:, ws], Sg1[:, ws], Sg2[:, ws])
                    nc.vector.tensor_scalar(
                        out=den[:, ws], in0=t1[:, ws], scalar1=0.5,
                        scalar2=float(D), op0=Alu.mult, op1=Alu.add)
                    nc.vector.tensor_scalar_max(den[:, ws], den[:, ws], 1.0)
                    nc.vector.tensor_mul(num[:, ws], thr[:, ws],
                                         den[:, ws])
                    nc.vector.tensor_add(num[:, ws], num[:, ws],
                                         R_t[:, ws])
                    nc.vector.reciprocal(rden[:, ws], den[:, ws])
                    nc.vector.tensor_mul(scale_a[:, ws], num[:, ws],
                                         rden[:, ws])
                    nc.vector.tensor_scalar_mul(scale_a[:, ws],
                                                scale_a[:, ws], 0.5)
                # transpose + codeT eviction
                for wi in tiles:
                    tp = tp_ps_pool.tile([P, 2 * D], FP8, tag="tp",
                                         name=f"tp_{wi}")
                    tp_v = tp[:].rearrange("p (n two) -> p n two",
                                           two=2)[:, :, 0]
                    for dt in range(16):
                        nc.tensor.transpose(
                            tp_v[:, dt * P:(dt + 1) * P],
                            code2s[wi][:, dt * P:(dt + 1) * P], eye_f8[:])
                    nc.vector.tensor_copy(
                        out=codeT_v[:, :, wi * P:(wi + 1) * P],
                        in_=tp_v.rearrange("p (dt o) -> p dt o", dt=16))
                # GEMM + evictions for this wave's o-tiles
                for j in tiles:
                    gemm_otile(j)

    nc.compile()
    return nc


def _get_nc(with_bias: bool):
    key = with_bias
    if key not in _CACHE:
        _CACHE[key] = _build(with_bias)
    return _CACHE[key]


def _build_in_maps(x: np.ndarray, W: np.ndarray, b: np.ndarray,
                   with_bias: bool):
    xf = x.reshape(B * S, D)
    eye = np.eye(P, dtype=np.float32)
    in_maps = []
    for c in range(N_CORES):
        m = {
            "xT": np.ascontiguousarray(xf[c * T:(c + 1) * T].T).astype(
                NP_BF16),
            "W": W,
            "eye": eye,
        }
        if with_bias:
            m["b"] = b
        in_maps.append(m)
    return in_maps


def kernel(x: np.ndarray, W: np.ndarray, b: np.ndarray) -> np.ndarray:
    x = np.asarray(x, dtype=np.float32)
    W = np.ascontiguousarray(np.asarray(W, dtype=np.float32))
    b = np.asarray(b, dtype=np.float32)
    with_bias = bool(np.any(b))

    nc = _get_nc(with_bias)
    in_maps = _build_in_maps(x, W, b, with_bias)
    res = run_bass_kernel_spmd(nc, in_maps, core_ids=list(range(N_CORES)))
    y = np.concatenate(
        [res.results[c]["yT"].astype(np.float32).T for c in range(N_CORES)],
        axis=0)
    return np.ascontiguousarray(y.reshape(B, S, O))


if __name__ == "__main__":
    rng = np.random.default_rng(0)
    x = rng.standard_normal((B, S, D), dtype=np.float32)
    W = (rng.standard_normal((O, D)) * 0.03).astype(np.float32)
    b = np.zeros((O,), dtype=np.float32)
    y = kernel(x, W, b)
    print("kernel ran, y shape", y.shape, "mean|y|", np.abs(y).mean())
